# revision 1
# baseline (speedup 1.0000x reference)
"""Trainium2 Bass kernel for nn_KnowledgeDifficulty.

Math (per batch b):
  logits = X[b] @ Wa + ba            (N, M)
  w      = softmax(logits, axis=N)   -- ba is constant along N => cancels
  d      = sigmoid(einsum(mL,L->m, einsum(nL,nm->mL, X[b], w), Ws) + bs)
         = sigmoid((sum_n e[n,m] * y[n]) / (sum_n e[n,m]) + bs)
    where e = exp(logits), y = X[b] @ Ws
  out    = d * (K > 0)

So the big (B,M,L) intermediate is never needed: only two PE passes over
the (N,M) logits (produce + weighted-reduce) plus one exp pass on ACT.

Sharding: data-parallel over B across 8 cores (8 batches/core).
Host prep: X is pre-transposed to XT[b] = X[b].T (L,N) so that matmul-1's
stationary operand needs no on-device transpose. Weights replicated.
"""

import numpy as np

B, N, L, M = 64, 512, 128, 1024
NCORES = 8
BLOC = B // NCORES  # 8 batches per core
NCH = N // 128  # 4 chunks of 128 along N
HALF = 512  # fp32 matmul max free dim (one PSUM bank)

_STATE = {}


def _build():
    import concourse.bacc as bacc
    import concourse.tile as tile
    import concourse.mybir as mybir

    f32 = mybir.dt.float32
    i32 = mybir.dt.int32
    Exp = mybir.ActivationFunctionType.Exp

    nc = bacc.Bacc(
        "TRN2", target_bir_lowering=False, debug=False, num_devices=NCORES
    )
    xt_d = nc.dram_tensor("xt", (BLOC, L, N), f32, kind="ExternalInput")
    wa_d = nc.dram_tensor("wa", (L, M), f32, kind="ExternalInput")
    ws_d = nc.dram_tensor("ws", (L, 1), f32, kind="ExternalInput")
    k_d = nc.dram_tensor("kk", (BLOC, M), i32, kind="ExternalInput")
    bn_d = nc.dram_tensor("bsneg", (BLOC, 1), f32, kind="ExternalInput")
    out_d = nc.dram_tensor("out", (BLOC, M), f32, kind="ExternalOutput")

    with tile.TileContext(nc) as tc:
        with (
            tc.tile_pool(name="const", bufs=1) as constp,
            tc.tile_pool(name="xtp", bufs=1) as xtp,
            tc.tile_pool(name="ep", bufs=6) as ep,
            tc.tile_pool(name="tsp", bufs=2) as tsp,
            tc.tile_pool(name="finp", bufs=1) as finp,
            tc.tile_pool(name="lgp", bufs=2, space="PSUM") as lgp,
            tc.tile_pool(name="ypp", bufs=2, space="PSUM") as ypp,
            tc.tile_pool(name="o2p", bufs=1, space="PSUM") as o2p,
        ):
            # ---- loads (all resident) ----
            xt_sb = xtp.tile([L, BLOC, N], f32)  # 16KB/partition
            nc.sync.dma_start(xt_sb[:], xt_d[:].rearrange("b p n -> p b n"))
            wa_sb = constp.tile([L, M], f32)
            nc.sync.dma_start(wa_sb[:], wa_d[:])
            ws_sb = constp.tile([L, 1], f32)
            nc.sync.dma_start(ws_sb[:], ws_d[:])
            k_sb = constp.tile([BLOC, M], i32)
            nc.sync.dma_start(k_sb[:], k_d[:])
            bn_sb = constp.tile([BLOC, 1], f32)
            nc.sync.dma_start(bn_sb[:], bn_d[:])

            # y2all: cols 0..31 hold y (one col per (b,chunk)), cols 32..63 all 1.0.
            # mm2 lhsT for (b,c) = strided pair [y_col, ones_col] (step 32).
            y2all = constp.tile([L, 2 * NCH * BLOC], f32)
            nc.vector.memset(y2all[:, NCH * BLOC : 2 * NCH * BLOC], 1.0)
            y2v = y2all[:].rearrange("p (two k) -> p k two", two=2)

            tall = finp.tile([BLOC, M], f32)
            sall = finp.tile([BLOC, M], f32)

            for b in range(BLOC):
                ypsum = ypp.tile([128, NCH], f32, tag="ypsum")
                es = []
                for c in range(NCH):
                    xt_c = xt_sb[:, b, c * 128 : (c + 1) * 128]
                    lg = lgp.tile([128, M], f32, tag="lg")
                    nc.tensor.matmul(lg[:, 0:HALF], xt_c, wa_sb[:, 0:HALF])
                    nc.tensor.matmul(lg[:, HALF:M], xt_c, wa_sb[:, HALF:M])
                    nc.tensor.matmul(ypsum[:, c : c + 1], xt_c, ws_sb[:])
                    e_c = ep.tile([128, M], f32, tag="e")
                    nc.scalar.activation(e_c[:], lg[:], Exp)
                    es.append(e_c)
                # y columns for this batch -> y2all[:, 4b:4b+4]
                nc.vector.tensor_copy(
                    y2all[:, b * NCH : (b + 1) * NCH], ypsum[:]
                )
                out2 = o2p.tile([2, M], f32, tag="out2")
                for h in range(2):
                    for c in range(NCH):
                        y2c = y2v[:, b * NCH + c, :]  # [128, 2] (y, 1)
                        nc.tensor.matmul(
                            out2[:, h * HALF : (h + 1) * HALF],
                            y2c,
                            es[c][:, h * HALF : (h + 1) * HALF],
                            start=(c == 0),
                            stop=(c == NCH - 1),
                        )
                ts_b = tsp.tile([2, M], f32, tag="ts")
                nc.vector.tensor_copy(ts_b[:], out2[:])
                nc.sync.dma_start(tall[b : b + 1, :], ts_b[0:1, :])
                nc.sync.dma_start(sall[b : b + 1, :], ts_b[1:2, :])

            # ---- epilogue: d = 1/(1+exp(-(t/s + bs))) * (K>0) ----
            recs = finp.tile([BLOC, M], f32)
            nc.vector.reciprocal(recs[:], sall[:])
            r = finp.tile([BLOC, M], f32)
            nc.vector.tensor_mul(r[:], tall[:], recs[:])
            u = finp.tile([BLOC, M], f32)
            nc.scalar.activation(u[:], r[:], Exp, bias=bn_sb[:], scale=-1.0)
            up1 = finp.tile([BLOC, M], f32)
            nc.vector.tensor_scalar_add(up1[:], u[:], 1.0)
            dd = finp.tile([BLOC, M], f32)
            nc.vector.reciprocal(dd[:], up1[:])
            kf = finp.tile([BLOC, M], f32)
            nc.vector.tensor_copy(kf[:], k_sb[:])
            dm = finp.tile([BLOC, M], f32)
            nc.vector.tensor_mul(dm[:], dd[:], kf[:])
            nc.sync.dma_start(out_d[:], dm[:])

    nc.compile()
    return nc


def _get_nc():
    if "nc" not in _STATE:
        _STATE["nc"] = _build()
    return _STATE["nc"]


def _make_in_maps(X, K, Wa, Ws, bs):
    X = np.ascontiguousarray(np.asarray(X, dtype=np.float32))
    K = np.ascontiguousarray(np.asarray(K, dtype=np.int32))
    Wa = np.ascontiguousarray(np.asarray(Wa, dtype=np.float32))
    Ws = np.ascontiguousarray(
        np.asarray(Ws, dtype=np.float32).reshape(L, 1)
    )
    bsv = float(np.asarray(bs, dtype=np.float32).reshape(-1)[0])
    bsneg = np.full((BLOC, 1), -bsv, dtype=np.float32)
    XT = np.ascontiguousarray(np.transpose(X, (0, 2, 1)))  # (B, L, N)
    in_maps = []
    for c in range(NCORES):
        sl = slice(c * BLOC, (c + 1) * BLOC)
        in_maps.append(
            dict(
                xt=np.ascontiguousarray(XT[sl]),
                wa=Wa,
                ws=Ws,
                kk=np.ascontiguousarray(K[sl]),
                bsneg=bsneg,
            )
        )
    return in_maps


def _run(X, K, Wa, Ws, bs, **spmd_kwargs):
    from concourse.bass_utils import run_bass_kernel_spmd

    nc = _get_nc()
    in_maps = _make_in_maps(X, K, Wa, Ws, bs)
    res = run_bass_kernel_spmd(
        nc, in_maps, core_ids=list(range(NCORES)), **spmd_kwargs
    )
    out = np.concatenate([r["out"] for r in res.results], axis=0)
    return np.ascontiguousarray(out.astype(np.float32)), res


def kernel(X, K, Wa, ba, Ws, bs):
    out, _ = _run(X, K, Wa, Ws, bs)
    return out


def kernel_traced(X, K, Wa, ba, Ws, bs):
    """Like kernel() but asks for an NTFF trace; returns (out, results)."""
    out, res = _run(X, K, Wa, Ws, bs, trace=True)
    return out, res


# revision 2
# speedup vs baseline: 2.4171x; 2.4171x over previous
"""Trainium2 Bass kernel for nn_KnowledgeDifficulty.

Math (per batch b):
  logits = X[b] @ Wa + ba            (N, M)
  w      = softmax(logits, axis=N)   -- ba is constant along N => cancels
  d      = sigmoid((sum_n e[n,m] * y[n]) / (sum_n e[n,m]) + bs)
    where e = exp(logits), y = X[b] @ Ws
  out    = d * (K > 0)

The (B,M,L) intermediate is never materialized: two PE passes over the
(N,M) logits (produce + weighted-reduce with lhsT=[y|1]) plus one exp
pass on ACT. PE matmuls run in bf16 (fp32 matmul double-pumps on trn2);
accumulation stays fp32 in PSUM, softmax ratio in fp32.

Sharding: data-parallel over B across 8 cores (8 batches/core).
Host prep: X pre-transposed (XT[b] = X[b].T) and cast to bf16 so
matmul-1's stationary operand needs no on-device transpose.
"""

import numpy as np

B, N, L, M = 64, 512, 128, 1024
NCORES = 8
BLOC = B // NCORES  # 8 batches per core
NCH = N // 128  # 4 chunks of 128 along N
HALF = 512  # one PSUM bank of fp32
FPB = M // 128  # 8 cols per batch in the [128, 64] epilogue layout

_STATE = {}


def _build():
    import concourse.bacc as bacc
    import concourse.tile as tile
    import concourse.mybir as mybir

    f32 = mybir.dt.float32
    bf16 = mybir.dt.bfloat16
    i32 = mybir.dt.int32
    Exp = mybir.ActivationFunctionType.Exp

    nc = bacc.Bacc(
        "TRN2", target_bir_lowering=False, debug=False, num_devices=NCORES
    )
    xt_d = nc.dram_tensor("xt", (BLOC, L, N), bf16, kind="ExternalInput")
    wa_d = nc.dram_tensor("wa", (L, M), bf16, kind="ExternalInput")
    ws_d = nc.dram_tensor("ws", (L, 1), bf16, kind="ExternalInput")
    k_d = nc.dram_tensor("kk", (BLOC, M), i32, kind="ExternalInput")
    bn_d = nc.dram_tensor("bsneg", (128, 1), f32, kind="ExternalInput")
    out_d = nc.dram_tensor("out", (BLOC, M), f32, kind="ExternalOutput")

    with tile.TileContext(nc) as tc:
        with (
            tc.tile_pool(name="const", bufs=1) as constp,
            tc.tile_pool(name="xtp", bufs=1) as xtp,
            tc.tile_pool(name="ep", bufs=6) as ep,
            tc.tile_pool(name="tsp", bufs=2) as tsp,
            tc.tile_pool(name="finp", bufs=1) as finp,
            tc.tile_pool(name="lgp", bufs=2, space="PSUM") as lgp,
            tc.tile_pool(name="ypp", bufs=2, space="PSUM") as ypp,
            tc.tile_pool(name="o2p", bufs=1, space="PSUM") as o2p,
        ):
            # ---- loads (all resident) ----
            xt_sb = xtp.tile([L, BLOC, N], bf16)  # 8KB/partition
            nc.sync.dma_start(xt_sb[:], xt_d[:].rearrange("b p n -> p b n"))
            wa_sb = constp.tile([L, M], bf16)
            nc.sync.dma_start(wa_sb[:], wa_d[:])
            ws_sb = constp.tile([L, 1], bf16)
            nc.sync.dma_start(ws_sb[:], ws_d[:])
            # K reshaped to the [128, b, f] epilogue layout (m = p*FPB + f)
            k_sb = constp.tile([128, BLOC, FPB], i32)
            nc.sync.dma_start(
                k_sb[:], k_d[:].rearrange("b (p f) -> p b f", p=128)
            )
            bn_sb = constp.tile([128, 1], f32)
            nc.sync.dma_start(bn_sb[:], bn_d[:])

            # y2all: cols 0..31 hold y (one col per (b,chunk)), cols 32..63 = 1.0.
            # mm2 lhsT for (b,c) = strided pair [y_col, ones_col] (step 32).
            y2all = constp.tile([L, 2 * NCH * BLOC], bf16)
            nc.vector.memset(y2all[:, NCH * BLOC : 2 * NCH * BLOC], 1.0)
            y2v = y2all[:].rearrange("p (two k) -> p k two", two=2)

            tall = finp.tile([128, BLOC, FPB], f32)
            sall = finp.tile([128, BLOC, FPB], f32)

            for b in range(BLOC):
                ypsum = ypp.tile([128, NCH], f32, tag="ypsum")
                es = []
                for c in range(NCH):
                    xt_c = xt_sb[:, b, c * 128 : (c + 1) * 128]
                    lg = lgp.tile([128, M], f32, tag="lg")
                    nc.tensor.matmul(lg[:, 0:HALF], xt_c, wa_sb[:, 0:HALF])
                    nc.tensor.matmul(lg[:, HALF:M], xt_c, wa_sb[:, HALF:M])
                    nc.tensor.matmul(ypsum[:, c : c + 1], xt_c, ws_sb[:])
                    e_c = ep.tile([128, M], bf16, tag="e")
                    nc.scalar.activation(e_c[:], lg[:], Exp)
                    es.append(e_c)
                # y columns for this batch -> y2all[:, 4b:4b+4] (f32->bf16)
                nc.vector.tensor_copy(
                    y2all[:, b * NCH : (b + 1) * NCH], ypsum[:]
                )
                out2 = o2p.tile([2, M], f32, tag="out2")
                for h in range(2):
                    for c in range(NCH):
                        y2c = y2v[:, b * NCH + c, :]  # [128, 2] (y, 1)
                        nc.tensor.matmul(
                            out2[:, h * HALF : (h + 1) * HALF],
                            y2c,
                            es[c][:, h * HALF : (h + 1) * HALF],
                            start=(c == 0),
                            stop=(c == NCH - 1),
                        )
                ts_b = tsp.tile([2, M], f32, tag="ts")
                nc.vector.tensor_copy(ts_b[:], out2[:])
                # scatter t/s rows into the [128, b, f] layout (m = p*FPB + f)
                nc.sync.dma_start(
                    tall[:, b, :],
                    ts_b[0:1, :].rearrange("one (p f) -> one p f", p=128),
                )
                nc.sync.dma_start(
                    sall[:, b, :],
                    ts_b[1:2, :].rearrange("one (p f) -> one p f", p=128),
                )

            # ---- epilogue: d = 1/(1+exp(-(t/s + bs))) * (K>0), [128, 64] ----
            W = BLOC * FPB
            tv = tall[:].rearrange("p b f -> p (b f)")
            sv = sall[:].rearrange("p b f -> p (b f)")
            recs = finp.tile([128, W], f32)
            nc.vector.reciprocal(recs[:], sv)
            r = finp.tile([128, W], f32)
            nc.vector.tensor_mul(r[:], tv, recs[:])
            u = finp.tile([128, W], f32)
            nc.scalar.activation(u[:], r[:], Exp, bias=bn_sb[:], scale=-1.0)
            up1 = finp.tile([128, W], f32)
            nc.vector.tensor_scalar_add(up1[:], u[:], 1.0)
            dd = finp.tile([128, W], f32)
            nc.vector.reciprocal(dd[:], up1[:])
            kf = finp.tile([128, W], f32)
            nc.vector.tensor_copy(kf[:], k_sb[:].rearrange("p b f -> p (b f)"))
            dm = finp.tile([128, BLOC, FPB], f32)
            nc.vector.tensor_mul(
                dm[:].rearrange("p b f -> p (b f)"), dd[:], kf[:]
            )
            nc.sync.dma_start(
                out_d[:].rearrange("b (p f) -> p b f", p=128), dm[:]
            )

    nc.compile()
    return nc


def _get_nc():
    if "nc" not in _STATE:
        _STATE["nc"] = _build()
    return _STATE["nc"]


def _make_in_maps(X, K, Wa, Ws, bs):
    import ml_dtypes

    bf16 = ml_dtypes.bfloat16
    X = np.asarray(X, dtype=np.float32)
    K = np.ascontiguousarray(np.asarray(K, dtype=np.int32))
    Wa = np.ascontiguousarray(np.asarray(Wa, dtype=np.float32).astype(bf16))
    Ws = np.ascontiguousarray(
        np.asarray(Ws, dtype=np.float32).astype(bf16).reshape(L, 1)
    )
    bsv = float(np.asarray(bs, dtype=np.float32).reshape(-1)[0])
    bsneg = np.full((128, 1), -bsv, dtype=np.float32)
    XT = np.ascontiguousarray(np.transpose(X, (0, 2, 1)).astype(bf16))
    in_maps = []
    for c in range(NCORES):
        sl = slice(c * BLOC, (c + 1) * BLOC)
        in_maps.append(
            dict(
                xt=np.ascontiguousarray(XT[sl]),
                wa=Wa,
                ws=Ws,
                kk=np.ascontiguousarray(K[sl]),
                bsneg=bsneg,
            )
        )
    return in_maps


def _run(X, K, Wa, Ws, bs, **spmd_kwargs):
    from concourse.bass_utils import run_bass_kernel_spmd

    nc = _get_nc()
    in_maps = _make_in_maps(X, K, Wa, Ws, bs)
    res = run_bass_kernel_spmd(
        nc, in_maps, core_ids=list(range(NCORES)), **spmd_kwargs
    )
    out = np.concatenate([r["out"] for r in res.results], axis=0)
    return np.ascontiguousarray(out.astype(np.float32)), res


def kernel(X, K, Wa, ba, Ws, bs):
    out, _ = _run(X, K, Wa, Ws, bs)
    return out


def kernel_traced(X, K, Wa, ba, Ws, bs):
    out, res = _run(X, K, Wa, Ws, bs, trace=False)
    return out, res


# revision 4
# speedup vs baseline: 2.5122x; 1.0393x over previous
"""Trainium2 Bass kernel for nn_KnowledgeDifficulty.

Math (per batch b):
  logits = X[b] @ Wa + ba            (N, M)
  w      = softmax(logits, axis=N)   -- ba is constant along N => cancels
  d      = sigmoid((sum_n e[n,m] * y[n]) / (sum_n e[n,m]) + bs)
    where e = exp(logits), y = X[b] @ Ws
  out    = d * (K > 0)

Two PE passes over the (N,M) logits (produce + weighted-reduce with
lhsT=[y|1]) plus one exp pass on ACT. Matmuls in bf16 (fp32 matmul
double-pumps on trn2), fp32 PSUM accumulation, fp32 softmax ratio.
mm2 packs 4 batches into the 4 32-col PE groups (tile_position col
tiling) so their streams overlap.

Sharding: data-parallel over B across 8 cores (8 batches/core).
Host prep: X pre-transposed+bf16; Ws fused as an extra Wa column;
bs/K fused in one int32 tensor; output returned in [128, b, f] layout
(host un-shuffles).
"""

import numpy as np

B, N, L, M = 64, 512, 128, 1024
NCORES = 8
BLOC = B // NCORES  # 8 batches per core
NCH = N // 128  # 4 chunks of 128 along N
HALF = 512  # one PSUM bank of fp32
FPB = M // 128  # 8 cols per batch in the [128, b, f] epilogue layout
NGRP = 2  # two groups of 4 batches (4 PE column groups each)
GSZ = BLOC // NGRP  # 4

_STATE = {}


def _build():
    import concourse.bacc as bacc
    import concourse.tile as tile
    import concourse.mybir as mybir

    f32 = mybir.dt.float32
    bf16 = mybir.dt.bfloat16
    i32 = mybir.dt.int32
    Exp = mybir.ActivationFunctionType.Exp

    nc = bacc.Bacc(
        "TRN2", target_bir_lowering=False, debug=False, num_devices=NCORES
    )
    # waws = [Wa | Ws | pad] (L, M+2)
    waws_d = nc.dram_tensor("waws", (L, M + 2), bf16, kind="ExternalInput")
    xt_d = nc.dram_tensor("xt", (BLOC, L, N), bf16, kind="ExternalInput")
    # bnk = [(-bs).f32-bits | K in [128, b, f] layout] (128, 1 + BLOC*FPB)
    bnk_d = nc.dram_tensor(
        "bnk", (128, 1 + BLOC * FPB), i32, kind="ExternalInput"
    )
    out_d = nc.dram_tensor("out", (128, BLOC, FPB), f32, kind="ExternalOutput")

    with tile.TileContext(nc) as tc:
        with (
            tc.tile_pool(name="const", bufs=1) as constp,
            tc.tile_pool(name="xtp", bufs=1) as xtp,
            tc.tile_pool(name="ep", bufs=20) as ep,
            tc.tile_pool(name="tsp", bufs=2) as tsp,
            tc.tile_pool(name="finp", bufs=1) as finp,
            tc.tile_pool(name="lgp", bufs=2, space="PSUM") as lgp,
            tc.tile_pool(name="ypp", bufs=2, space="PSUM") as ypp,
            tc.tile_pool(name="o2p", bufs=1, space="PSUM") as o2p,
        ):
            # ---- loads (weights first; xt per batch, split across queues) ----
            waws_sb = constp.tile([L, M + 2], bf16)
            nc.sync.dma_start(waws_sb[:], waws_d[:])
            wa_sb = waws_sb[:, 0:M]
            ws_sb = waws_sb[:, M : M + 1]

            xt_sb = xtp.tile([L, BLOC, N], bf16)
            for b in range(BLOC):
                eng = nc.sync if b % 2 == 0 else nc.gpsimd
                eng.dma_start(xt_sb[:, b, :], xt_d[b])

            bnk_sb = constp.tile([128, 1 + BLOC * FPB], i32)
            nc.gpsimd.dma_start(bnk_sb[:], bnk_d[:])
            bn_sb = bnk_sb[:, 0:1].bitcast(f32)

            # y2all: cols 0..31 hold y (one col per (b,chunk)), cols 32..63 = 1.0
            y2all = constp.tile([L, 2 * NCH * BLOC], bf16)
            nc.vector.memset(y2all[:, NCH * BLOC : 2 * NCH * BLOC], 1.0)
            y2v = y2all[:].rearrange("p (two k) -> p k two", two=2)

            for g in range(NGRP):
                es = {}
                for j in range(GSZ):
                    b = g * GSZ + j
                    ypsum = ypp.tile([128, NCH], f32, tag="ypsum")
                    for c in range(NCH):
                        xt_c = xt_sb[:, b, c * 128 : (c + 1) * 128]
                        lg = lgp.tile([128, M], f32, tag="lg")
                        nc.tensor.matmul(lg[:, 0:HALF], xt_c, wa_sb[:, 0:HALF])
                        nc.tensor.matmul(lg[:, HALF:M], xt_c, wa_sb[:, HALF:M])
                        nc.tensor.matmul(ypsum[:, c : c + 1], xt_c, ws_sb)
                        e_c = ep.tile([128, M], bf16, tag="e")
                        nc.scalar.activation(e_c[:], lg[:], Exp)
                        es[(j, c)] = e_c
                    nc.vector.tensor_copy(
                        y2all[:, b * NCH : (b + 1) * NCH], ypsum[:]
                    )
                # mm2: 4 batches concurrently in the 4 PE column groups
                out2 = o2p.tile([128, M], f32, tag="out2")
                for h in range(2):
                    for c in range(NCH):
                        for j in range(GSZ):
                            b = g * GSZ + j
                            nc.tensor.matmul(
                                out2[
                                    32 * j : 32 * j + 2,
                                    h * HALF : (h + 1) * HALF,
                                ],
                                y2v[:, b * NCH + c, :],
                                es[(j, c)][:, h * HALF : (h + 1) * HALF],
                                start=(c == 0),
                                stop=(c == NCH - 1),
                                skip_group_check=True,
                                tile_position=(0, 32 * j),
                            )
                ts_g = tsp.tile([128, M], f32, tag="ts")
                nc.vector.tensor_copy(ts_g[:], out2[:])

                # scatter t/s rows (32j, 32j+1) into [128, b, f] layout
                tall = finp.tile([128, GSZ, FPB], f32, tag=f"tall{g}")
                sall = finp.tile([128, GSZ, FPB], f32, tag=f"sall{g}")
                for j in range(GSZ):
                    te = nc.sync if j % 2 == 0 else nc.gpsimd
                    te.dma_start(
                        tall[:, j, :],
                        ts_g[32 * j : 32 * j + 1, :].rearrange(
                            "one (p f) -> one p f", p=128
                        ),
                    )
                    te.dma_start(
                        sall[:, j, :],
                        ts_g[32 * j + 1 : 32 * j + 2, :].rearrange(
                            "one (p f) -> one p f", p=128
                        ),
                    )

                # per-group epilogue: d = 1/(1+exp(-(t/s + bs))) * (K>0)
                W = GSZ * FPB  # 32
                tv = tall[:].rearrange("p j f -> p (j f)")
                sv = sall[:].rearrange("p j f -> p (j f)")
                recs = finp.tile([128, W], f32, tag=f"recs{g}")
                nc.vector.reciprocal(recs[:], sv)
                r = finp.tile([128, W], f32, tag=f"r{g}")
                nc.vector.tensor_mul(r[:], tv, recs[:])
                u = finp.tile([128, W], f32, tag=f"u{g}")
                nc.scalar.activation(u[:], r[:], Exp, bias=bn_sb, scale=-1.0)
                up1 = finp.tile([128, W], f32, tag=f"up1{g}")
                nc.vector.tensor_scalar_add(up1[:], u[:], 1.0)
                dd = finp.tile([128, W], f32, tag=f"dd{g}")
                nc.vector.reciprocal(dd[:], up1[:])
                kf = finp.tile([128, W], f32, tag=f"kf{g}")
                nc.vector.tensor_copy(
                    kf[:], bnk_sb[:, 1 + g * W : 1 + (g + 1) * W]
                )
                dm = finp.tile([128, GSZ, FPB], f32, tag=f"dm{g}")
                nc.vector.tensor_mul(
                    dm[:].rearrange("p j f -> p (j f)"), dd[:], kf[:]
                )
                nc.sync.dma_start(out_d[:, g * GSZ : (g + 1) * GSZ, :], dm[:])

    nc.compile()
    return nc


def _get_nc():
    if "nc" not in _STATE:
        _STATE["nc"] = _build()
    return _STATE["nc"]


def _make_in_maps(X, K, Wa, Ws, bs):
    import ml_dtypes

    bf16 = ml_dtypes.bfloat16
    X = np.asarray(X, dtype=np.float32)
    K = np.ascontiguousarray(np.asarray(K, dtype=np.int32))
    Wa = np.asarray(Wa, dtype=np.float32)
    Ws = np.asarray(Ws, dtype=np.float32)
    bsv = float(np.asarray(bs, dtype=np.float32).reshape(-1)[0])

    waws = np.zeros((L, M + 2), dtype=bf16)
    waws[:, 0:M] = Wa.astype(bf16)
    waws[:, M] = Ws.astype(bf16)
    XT = np.ascontiguousarray(np.transpose(X, (0, 2, 1)).astype(bf16))

    bneg = np.full((128, 1), -bsv, dtype=np.float32)
    in_maps = []
    for c in range(NCORES):
        sl = slice(c * BLOC, (c + 1) * BLOC)
        # K[b, m] with m = p*FPB + f  ->  k128[p, b, f]
        k128 = (
            K[sl].reshape(BLOC, 128, FPB).transpose(1, 0, 2).reshape(128, -1)
        )
        bnk = np.concatenate(
            [bneg.view(np.int32), np.ascontiguousarray(k128)], axis=1
        )
        in_maps.append(
            dict(
                xt=np.ascontiguousarray(XT[sl]),
                waws=waws,
                bnk=np.ascontiguousarray(bnk),
            )
        )
    return in_maps


def _run(X, K, Wa, Ws, bs, **spmd_kwargs):
    from concourse.bass_utils import run_bass_kernel_spmd

    nc = _get_nc()
    in_maps = _make_in_maps(X, K, Wa, Ws, bs)
    res = run_bass_kernel_spmd(
        nc, in_maps, core_ids=list(range(NCORES)), **spmd_kwargs
    )
    outs = []
    for r in res.results:
        o = r["out"]  # (128, BLOC, FPB): out[p, b, f] = result[b, p*FPB+f]
        outs.append(np.transpose(o, (1, 0, 2)).reshape(BLOC, M))
    return np.ascontiguousarray(
        np.concatenate(outs, axis=0).astype(np.float32)
    ), res


def kernel(X, K, Wa, ba, Ws, bs):
    out, _ = _run(X, K, Wa, Ws, bs)
    return out


def kernel_traced(X, K, Wa, ba, Ws, bs):
    out, res = _run(X, K, Wa, Ws, bs, trace=False)
    return out, res


# revision 9
# speedup vs baseline: 2.6697x; 1.0627x over previous
"""Trainium2 Bass kernel for nn_KnowledgeDifficulty.

Math (per batch b):
  logits = X[b] @ Wa + ba            (N, M)
  w      = softmax(logits, axis=N)   -- ba is constant along N => cancels
  d      = sigmoid((sum_n e[n,m] * y[n]) / (sum_n e[n,m]) + bs)
    where e = exp(logits), y = X[b] @ Ws
  out    = d * (K > 0)

Two PE passes over the (N,M) logits (produce + weighted-reduce with
lhsT=[y|1]) plus one exp pass on ACT. Matmuls in bf16 (fp32 matmul
double-pumps on trn2), fp32 PSUM accumulation, fp32 softmax ratio.
mm2 packs 4 batches into the 4 32-col PE groups (tile_position col
tiling) so their streams overlap.

Sharding: data-parallel over B across 8 cores (8 batches/core).
Host prep: X pre-transposed+bf16; Ws fused as an extra Wa column;
bs/K fused in one int32 tensor; output returned in [128, b, f] layout
(host un-shuffles).
"""

import numpy as np

B, N, L, M = 64, 512, 128, 1024
NCORES = 8
BLOC = B // NCORES  # 8 batches per core
NCH = N // 128  # 4 chunks of 128 along N
HALF = 512  # one PSUM bank of fp32
FPB = M // 128  # 8 cols per batch in the [128, b, f] epilogue layout
NGRP = 2  # two groups of 4 batches (4 PE column groups each)
GSZ = BLOC // NGRP  # 4

_STATE = {}


def _build():
    import concourse.bacc as bacc
    import concourse.tile as tile
    import concourse.mybir as mybir

    f32 = mybir.dt.float32
    bf16 = mybir.dt.bfloat16
    i32 = mybir.dt.int32
    Exp = mybir.ActivationFunctionType.Exp

    nc = bacc.Bacc(
        "TRN2", target_bir_lowering=False, debug=False, num_devices=NCORES
    )
    # waws = [Wa | Ws | pad] (L, M+2)
    waws_d = nc.dram_tensor("waws", (L, M + 2), bf16, kind="ExternalInput")
    xt_d = nc.dram_tensor("xt", (BLOC, L, N), bf16, kind="ExternalInput")
    # bnk = [(-bs).f32-bits | K in [128, b, f] layout] (128, 1 + BLOC*FPB)
    bnk_d = nc.dram_tensor(
        "bnk", (128, 1 + BLOC * FPB), i32, kind="ExternalInput"
    )
    out_d = nc.dram_tensor("out", (128, BLOC, FPB), f32, kind="ExternalOutput")

    with tile.TileContext(nc) as tc:
        with (
            tc.tile_pool(name="const", bufs=1) as constp,
            tc.tile_pool(name="xtp", bufs=1) as xtp,
            tc.tile_pool(name="ep", bufs=34) as ep,
            tc.tile_pool(name="tsp", bufs=2) as tsp,
            tc.tile_pool(name="finp", bufs=1) as finp,
            tc.tile_pool(name="lgp", bufs=2, space="PSUM") as lgp,
            tc.tile_pool(name="ypp", bufs=2, space="PSUM") as ypp,
            tc.tile_pool(name="o2p", bufs=1, space="PSUM") as o2p,
        ):
            # ---- loads (weights first; xt per batch, split across queues) ----
            waws_sb = constp.tile([L, M + 2], bf16)
            nc.sync.dma_start(waws_sb[:], waws_d[:])
            wa_sb = waws_sb[:, 0:M]
            ws_sb = waws_sb[:, M : M + 1]

            xt_sb = xtp.tile([L, BLOC, N], bf16)
            for b in range(BLOC):
                eng = nc.gpsimd if b % 2 == 0 else nc.sync
                eng.dma_start(xt_sb[:, b, :], xt_d[b])

            bnk_sb = constp.tile([128, 1 + BLOC * FPB], i32)
            nc.sync.dma_start(bnk_sb[:], bnk_d[:])
            bn_sb = bnk_sb[:, 0:1].bitcast(f32)

            # y2all: cols 0..31 hold y (one col per (b,chunk)), cols 32..63 = 1.0
            y2all = constp.tile([L, 2 * NCH * BLOC], bf16)
            nc.vector.memset(y2all[:, NCH * BLOC : 2 * NCH * BLOC], 1.0)
            y2v = y2all[:].rearrange("p (two k) -> p k two", two=2)

            # hoist K->f32 masks out of the tail
            kfs = []
            for g in range(NGRP):
                W = GSZ * FPB
                kf = finp.tile([128, W], f32, tag=f"kf{g}", name=f"kf{g}")
                nc.vector.tensor_copy(
                    kf[:], bnk_sb[:, 1 + g * W : 1 + (g + 1) * W]
                )
                kfs.append(kf)

            # phase B: all logits + exp + y (keeps ACT saturated end to end)
            es = {}
            for b in range(BLOC):
                ypsum = ypp.tile([128, NCH], f32, tag="ypsum")
                for c in range(NCH):
                    xt_c = xt_sb[:, b, c * 128 : (c + 1) * 128]
                    lg = lgp.tile([128, M], f32, tag="lg")
                    nc.tensor.matmul(lg[:, 0:HALF], xt_c, wa_sb[:, 0:HALF])
                    nc.tensor.matmul(lg[:, HALF:M], xt_c, wa_sb[:, HALF:M])
                    nc.tensor.matmul(ypsum[:, c : c + 1], xt_c, ws_sb)
                    e_c = ep.tile([128, M], bf16, tag="e")
                    nc.scalar.activation(e_c[:], lg[:], Exp)
                    es[(b, c)] = e_c
                nc.vector.tensor_copy(
                    y2all[:, b * NCH : (b + 1) * NCH], ypsum[:]
                )

            # phase C: weighted reduce + epilogue per group of 4 batches
            for g in range(NGRP):
                # mm2: 4 batches concurrently in the 4 PE column groups
                out2 = o2p.tile([128, M], f32, tag="out2")
                for h in range(2):
                    for c in range(NCH):
                        for j in range(GSZ):
                            b = g * GSZ + j
                            nc.tensor.matmul(
                                out2[
                                    32 * j : 32 * j + 2,
                                    h * HALF : (h + 1) * HALF,
                                ],
                                y2v[:, b * NCH + c, :],
                                es[(b, c)][:, h * HALF : (h + 1) * HALF],
                                start=(c == 0),
                                stop=(c == NCH - 1),
                                skip_group_check=True,
                                tile_position=(0, 32 * j),
                            )
                ts_g = tsp.tile([128, M], f32, tag="ts")
                if g == NGRP - 1:
                    # ACT is done with exps by now and its PSUM port is fast
                    nc.scalar.copy(ts_g[:], out2[:])
                else:
                    nc.vector.tensor_copy(ts_g[:], out2[:])

                # scatter t/s rows (32j, 32j+1) into [128, b, f] layout;
                # s on sync, t on gpsimd so the recip can start off s alone
                tall = finp.tile([128, GSZ, FPB], f32, tag=f"tall{g}")
                sall = finp.tile([128, GSZ, FPB], f32, tag=f"sall{g}")
                for j in range(GSZ):
                    nc.sync.dma_start(
                        sall[:, j, :],
                        ts_g[32 * j + 1 : 32 * j + 2, :].rearrange(
                            "one (p f) -> one p f", p=128
                        ),
                    )
                    nc.gpsimd.dma_start(
                        tall[:, j, :],
                        ts_g[32 * j : 32 * j + 1, :].rearrange(
                            "one (p f) -> one p f", p=128
                        ),
                    )

                # per-group epilogue: d = 1/(1+exp(-(t/s + bs))) * (K>0)
                W = GSZ * FPB  # 32
                tv = tall[:].rearrange("p j f -> p (j f)")
                sv = sall[:].rearrange("p j f -> p (j f)")
                recs = finp.tile([128, W], f32, tag=f"recs{g}")
                nc.vector.reciprocal(recs[:], sv)
                r = finp.tile([128, W], f32, tag=f"r{g}")
                nc.vector.tensor_mul(r[:], tv, recs[:])
                u = finp.tile([128, W], f32, tag=f"u{g}")
                nc.scalar.activation(u[:], r[:], Exp, bias=bn_sb, scale=-1.0)
                up1 = finp.tile([128, W], f32, tag=f"up1{g}")
                nc.vector.tensor_scalar_add(up1[:], u[:], 1.0)
                dd = finp.tile([128, W], f32, tag=f"dd{g}")
                nc.vector.reciprocal(dd[:], up1[:])
                dm = finp.tile([128, GSZ, FPB], f32, tag=f"dm{g}")
                nc.vector.tensor_mul(
                    dm[:].rearrange("p j f -> p (j f)"), dd[:], kfs[g][:]
                )
                nc.sync.dma_start(out_d[:, g * GSZ : (g + 1) * GSZ, :], dm[:])

    nc.compile()
    return nc


def _get_nc():
    if "nc" not in _STATE:
        _STATE["nc"] = _build()
    return _STATE["nc"]


def _make_in_maps(X, K, Wa, Ws, bs):
    import ml_dtypes

    bf16 = ml_dtypes.bfloat16
    X = np.asarray(X, dtype=np.float32)
    K = np.ascontiguousarray(np.asarray(K, dtype=np.int32))
    Wa = np.asarray(Wa, dtype=np.float32)
    Ws = np.asarray(Ws, dtype=np.float32)
    bsv = float(np.asarray(bs, dtype=np.float32).reshape(-1)[0])

    waws = np.zeros((L, M + 2), dtype=bf16)
    waws[:, 0:M] = Wa.astype(bf16)
    waws[:, M] = Ws.astype(bf16)
    XT = np.ascontiguousarray(np.transpose(X, (0, 2, 1)).astype(bf16))

    bneg = np.full((128, 1), -bsv, dtype=np.float32)
    in_maps = []
    for c in range(NCORES):
        sl = slice(c * BLOC, (c + 1) * BLOC)
        # K[b, m] with m = p*FPB + f  ->  k128[p, b, f]
        k128 = (
            K[sl].reshape(BLOC, 128, FPB).transpose(1, 0, 2).reshape(128, -1)
        )
        bnk = np.concatenate(
            [bneg.view(np.int32), np.ascontiguousarray(k128)], axis=1
        )
        in_maps.append(
            dict(
                xt=np.ascontiguousarray(XT[sl]),
                waws=waws,
                bnk=np.ascontiguousarray(bnk),
            )
        )
    return in_maps


def _run(X, K, Wa, Ws, bs, **spmd_kwargs):
    from concourse.bass_utils import run_bass_kernel_spmd

    nc = _get_nc()
    in_maps = _make_in_maps(X, K, Wa, Ws, bs)
    res = run_bass_kernel_spmd(
        nc, in_maps, core_ids=list(range(NCORES)), **spmd_kwargs
    )
    outs = []
    for r in res.results:
        o = r["out"]  # (128, BLOC, FPB): out[p, b, f] = result[b, p*FPB+f]
        outs.append(np.transpose(o, (1, 0, 2)).reshape(BLOC, M))
    return np.ascontiguousarray(
        np.concatenate(outs, axis=0).astype(np.float32)
    ), res


def kernel(X, K, Wa, ba, Ws, bs):
    out, _ = _run(X, K, Wa, Ws, bs)
    return out


def kernel_traced(X, K, Wa, ba, Ws, bs):
    out, res = _run(X, K, Wa, Ws, bs, trace=False)
    return out, res
